# revision 12
# baseline (speedup 1.0000x reference)
"""2-layer GAT on 8 Trainium2 NeuronCores (Bass/Tile).

Sharding: dst nodes partitioned into 8 contiguous ranges (6250 each); each core
owns its range's incoming edges (host-sorted by dst). Dense projections are
computed redundantly per core into a per-core DRAM feature table; per-edge
source rows are fetched with dma_gather (int16 indices -> table split in two
halves). Segment softmax skips the max-subtraction (exactly cancels; the 1e-16
eps is negligible since every node has a self-loop). Aggregation: per 128-edge
chunk a one-hot S[p,m]=(dstloc[p]==m) is built on the vector engine and a PE
matmul S.T @ [u*h | u] accumulates numerator and denominator in PSUM. Layer-1
output (post bias+ELU) is transposed on-chip and AllGathered across the 8
cores; layer 2 repeats the same pipeline with one head.

kernel(**inputs) takes the full unsharded inputs and returns [50000, 64] f32.
"""
import numpy as np

import concourse.bass as bass
import concourse.bacc as bacc
import concourse.mybir as mybir
import concourse.tile as tile
from concourse import bass_utils

f32 = mybir.dt.float32
i16 = mybir.dt.int16
i32 = mybir.dt.int32
AF = mybir.ActivationFunctionType
ALU = mybir.AluOpType

NEG_SLOPE = 0.2

# problem geometry (graded problem)
N = 50000
F_IN = 128
H1, C1 = 4, 64
D1 = H1 * C1            # 256
C2 = 64
NCORES = 8
T1W = 320               # t1 row: h1[0:256], a_s1[256:260], a_d1[260:264], pad
T2W = 128               # t2 row: h2[0:64], a_s2[64], a_d2[65], pad
ADW = 64                # ad-table row width (256B = min gather granularity)
B = 32                  # chunks per batch (128 edges per chunk)
BIGIDX = 10**6


# ============================ host preprocessing ============================

def _preprocess(edge_index, n=N, ncores=NCORES):
    nloc = n // ncores
    split = n // 2
    src = np.asarray(edge_index[0], np.int64)
    dst = np.asarray(edge_index[1], np.int64)
    loop = np.arange(n, dtype=np.int64)
    src = np.concatenate([src, loop])
    dst = np.concatenate([dst, loop])
    order = np.argsort(dst, kind="stable")
    src, dst = src[order], dst[order]
    cuts = np.searchsorted(dst, np.arange(ncores + 1) * nloc)

    ntiles = -(-nloc // 128)
    per = [[None] * ntiles for _ in range(ncores)]
    for c in range(ncores):
        s, e = cuts[c], cuts[c + 1]
        cs, cd = src[s:e], dst[s:e] - c * nloc
        tcuts = np.searchsorted(cd, np.arange(ntiles + 1) * 128)
        for t in range(ntiles):
            a, b = tcuts[t], tcuts[t + 1]
            ts, td = cs[a:b], cd[a:b] - t * 128
            m = ts < split
            per[c][t] = (ts[m], td[m], ts[~m] - split, td[~m])

    # uniform per-(tile, side) chunk counts = max over cores
    nlo = [max(-(-len(per[c][t][0]) // 128) for c in range(ncores))
           for t in range(ntiles)]
    nhi = [max(-(-len(per[c][t][2]) // 128) for c in range(ncores))
           for t in range(ntiles)]

    seq = []
    for t in range(ntiles):
        seq += [(t, 0)] * nlo[t] + [(t, 1)] * nhi[t]
    nch = len(seq)
    nb = -(-nch // B)
    seq += [(-1, 0)] * (nb * B - nch)

    batches = []
    for bi in range(nb):
        blk = seq[bi * B:(bi + 1) * B]
        batches.append([x for x in blk if x[1] == 0] + [x for x in blk if x[1] == 1])
    k_b = [sum(1 for x in blk if x[1] == 0) for blk in batches]

    tile_slots = [[] for _ in range(ntiles)]
    for bi, blk in enumerate(batches):
        for s, (t, _) in enumerate(blk):
            if t >= 0:
                tile_slots[t].append((bi, s))

    def wrap(a):        # [128] -> [16, 8] -> replicate to [128, 8]
        return np.tile(a.reshape(-1, 16).T, (8, 1))

    idx1 = np.zeros((ncores, nb, 128, B * 8), np.int16)
    adidx = np.zeros((ncores, nb, 128, B * 8), np.int16)
    dstloc = np.full((ncores, nb, 128, B), -1.0, np.float32)

    for c in range(ncores):
        cur = {}
        for bi, blk in enumerate(batches):
            gall = np.zeros((B, 128), np.int16)
            aall = np.zeros((B, 128), np.int16)
            for s, (t, hi) in enumerate(blk):
                if t < 0:
                    continue
                j = cur.get((t, hi), 0)
                cur[(t, hi)] = j + 1
                lo_s, lo_d, hi_s, hi_d = per[c][t]
                es, ed = (hi_s, hi_d) if hi else (lo_s, lo_d)
                a, b2 = j * 128, min((j + 1) * 128, len(es))
                ln = max(0, b2 - a)
                if ln > 0:
                    gall[s, :ln] = es[a:b2]
                    aall[s, :ln] = ed[a:b2] + t * 128
                    dstloc[c, bi, :ln, s] = ed[a:b2]
            idx1[c, bi] = np.concatenate([wrap(g) for g in gall], axis=1)
            adidx[c, bi] = np.concatenate([wrap(a2) for a2 in aall], axis=1)

    # scatter-add indices for the local ad tables: per group of SG dense
    # tiles, 128*SG wrapped int16 indices; non-local nodes go to trash row
    # `nloc`. idx position i <-> (partition i%128, slot i//128).
    SG = 32
    dt_ = -(-n // 128)
    nsg = -(-dt_ // SG)
    adsc = np.full((ncores, nsg, 128, SG * 8), 0, np.int16)
    for c in range(ncores):
        for g in range(nsg):
            vals = np.full(SG * 128, nloc, np.int64)
            for sl in range(SG):
                t = g * SG + sl
                if t >= dt_:
                    continue
                nodes = np.arange(t * 128, t * 128 + 128)
                loc = nodes - c * nloc
                ok = (loc >= 0) & (loc < nloc) & (nodes < n)
                i0 = sl * 128
                vals[i0:i0 + 128][ok] = loc[ok]
            adsc[c, g] = np.tile(vals.reshape(-1, 16).T, (8, 1))

    meta = dict(nb=nb, k_b=k_b, batches=batches, tile_slots=tile_slots,
                ntiles=ntiles, nloc=nloc, n=n, split=split, dt=dt_,
                ncores=ncores, sg=SG, nsg=nsg)
    data = dict(idx1=idx1, adidx=adidx, dstloc=dstloc, adsc=adsc)
    return meta, data


# ============================ device program ============================

def _ap(t_ap, dims, off=0):
    return bass.AP(t_ap.tensor, t_ap.offset + off, dims)


def _build(meta):
    import os
    PH = int(os.environ.get("GAT_PHASES", "5"))
    n, nloc, split, ncores = meta["n"], meta["nloc"], meta["split"], meta["ncores"]
    nb, dt_ = meta["nb"], meta["dt"]

    nc = bacc.Bacc("TRN2", target_bir_lowering=False, debug=False,
                   num_devices=ncores)

    xT = nc.dram_tensor("xT", [F_IN, n], f32, kind="ExternalInput")
    w1c = nc.dram_tensor("w1c", [F_IN, D1 + 8], f32, kind="ExternalInput")
    w2c = nc.dram_tensor("w2c", [128, 2, C2 + 2], f32, kind="ExternalInput")
    b1r = nc.dram_tensor("b1r", [128, D1], f32, kind="ExternalInput")
    b2r = nc.dram_tensor("b2r", [128, C2], f32, kind="ExternalInput")
    iota_in = nc.dram_tensor("iota_in", [128, 128], f32, kind="ExternalInput")
    ident_in = nc.dram_tensor("ident_in", [128, 128], f32, kind="ExternalInput")
    idx1_in = nc.dram_tensor("idx1", [nb, 128, B * 8], i16, kind="ExternalInput")
    adidx_in = nc.dram_tensor("adidx", [nb, 128, B * 8], i16, kind="ExternalInput")
    dstl_in = nc.dram_tensor("dstl", [nb, 128, B], f32, kind="ExternalInput")
    adsc_in = nc.dram_tensor("adsc", [meta["nsg"], 128, meta["sg"] * 8], i16,
                             kind="ExternalInput")
    out_loc = nc.dram_tensor("out_local", [nloc, C2], f32, kind="ExternalOutput")

    t1 = nc.dram_tensor("t1", [n, T1W], f32, kind="Internal")
    t2 = nc.dram_tensor("t2", [n, T2W], f32, kind="Internal")
    ad1 = nc.dram_tensor("ad1", [nloc + 1, ADW], f32, kind="Internal")
    ad2 = nc.dram_tensor("ad2", [nloc + 1, ADW], f32, kind="Internal")
    from concourse.replica_groups import maybe_share_collective_output_space
    ag_space = maybe_share_collective_output_space(
        "AllGather", [list(range(ncores))])
    b1t = nc.dram_tensor("b1t", [2 * 128, nloc], f32, kind="Internal")
    ag = nc.dram_tensor("ag", [ncores * 2 * 128, nloc], f32, kind="Internal",
                        addr_space=ag_space)

    with tile.TileContext(nc) as tc:
        with (
            tc.tile_pool(name="const", bufs=1) as cp,
            tc.tile_pool(name="dense", bufs=2) as dp,
            tc.tile_pool(name="stag", bufs=3) as sp,
            tc.tile_pool(name="adstag", bufs=1) as ap_,
            tc.tile_pool(name="msg", bufs=2) as mp,
            tc.tile_pool(name="gath", bufs=2) as gp,
            tc.tile_pool(name="norm", bufs=2) as np_,
            tc.tile_pool(name="psd", bufs=2, space="PSUM") as psd,
            tc.tile_pool(name="psa", bufs=4, space="PSUM") as psa,
            tc.tile_pool(name="pst", bufs=2, space="PSUM") as pst,
        ):
            zeros = cp.tile([128, 64], f32, tag="zeros")
            nc.vector.memset(zeros[:], 0.0)
            # zero-fill ad tables (only cols 0:4 are written by the scatter;
            # the gather reads whole 64-col rows)
            for adt_ in (ad1, ad2):
                nfull = ((nloc + 1) // 128) * 128
                if nfull > 0:
                    nc.sync.dma_start(
                        adt_[:][0:nfull, :],
                        _ap(zeros[:], [[zeros[:].ap[0][0], 128],
                                       [0, nfull // 128], [1, ADW]]))
                if nloc + 1 > nfull:
                    nc.sync.dma_start(adt_[:][nfull:nloc + 1, :],
                                      zeros[0:nloc + 1 - nfull, 0:ADW])
            w1sb = cp.tile([F_IN, D1 + 8], f32, tag="w1")
            w2sb = cp.tile([128, 2, C2 + 2], f32, tag="w2")
            b1sb = cp.tile([128, D1], f32, tag="b1")
            b2sb = cp.tile([128, C2], f32, tag="b2")
            iota = cp.tile([128, 128], f32, tag="iota")
            ident = cp.tile([128, 128], f32, tag="ident")
            nc.sync.dma_start(w1sb[:], w1c[:])
            nc.sync.dma_start(w2sb[:], w2c[:])
            nc.sync.dma_start(b1sb[:], b1r[:])
            nc.sync.dma_start(b2sb[:], b2r[:])
            nc.sync.dma_start(iota[:], iota_in[:])
            nc.sync.dma_start(ident[:], ident_in[:])

            # =================== dense phase 1 ===================
            SG = meta["sg"]
            nsg = meta["nsg"]
            adst_cur = [None]

            def ad_group_start(g):
                adst = ap_.tile([128, SG, ADW], f32, tag="adst",
                                name=f"adst_{g}")
                nc.vector.memset(adst[:], 0.0)
                adst_cur[0] = adst

            def ad_group_end(g, adt_):
                six = gp.tile([128, SG * 8], i16, tag="scix")
                nc.sync.dma_start(six[:], adsc_in[:][g])
                nc.gpsimd.dma_scatter_add(
                    out_ap=adt_[:], in_ap=adst_cur[0][:], idxs_ap=six[:],
                    num_idxs=SG * 128, num_idxs_reg=SG * 128, elem_size=ADW,
                    single_packet=False)

            GD = 8
            for g in range(-(-dt_ // GD)):
                t0 = g * GD
                tn = min(GD, dt_ - t0)
                cols = min(tn * 128, n - t0 * 128)
                xt = dp.tile([F_IN, GD * 128], f32, tag="xt")
                nc.sync.dma_start(xt[:, 0:cols], xT[:, t0 * 128:t0 * 128 + cols])
                for t in range(t0, t0 + tn):
                    if t % SG == 0:
                        ad_group_start(t // SG)
                    m = min(128, n - t * 128)
                    ps = psd.tile([128, D1 + 8], f32, tag="psd")
                    nc.tensor.matmul(
                        ps[0:m, :], xt[:, (t - t0) * 128:(t - t0) * 128 + m],
                        w1sb[:], start=True, stop=True)
                    st = sp.tile([128, T1W], f32, tag="st1")
                    nc.gpsimd.memset(st[0:m, D1 + 8:T1W], 0.0)
                    nc.scalar.activation(st[0:m, 0:D1 + 8], ps[0:m, :], AF.Copy)
                    nc.vector.tensor_copy(adst_cur[0][0:m, t % SG, 0:4],
                                          ps[0:m, D1 + 4:D1 + 8])
                    nc.sync.dma_start(t1[:][t * 128:t * 128 + m, :], st[0:m, :])
                    if t % SG == SG - 1 or t == dt_ - 1:
                        ad_group_end(t // SG, ad1)

            # =================== layer-1 gather/aggregate ===================
            if PH >= 2:
                _gather_layer(nc, meta, mp, gp, np_, psa, pst, layer=1, tab=t1,
                              adt=ad1, idx_in=idx1_in, adidx_in=adidx_in,
                              dstl_in=dstl_in, iota=iota, ident=ident,
                              bias=b1sb, bounce=b1t, out_loc=None)

            # =================== AllGather ===================
            if PH >= 3:
                nc.gpsimd.collective_compute(
                    "AllGather", ALU.bypass,
                    replica_groups=[list(range(ncores))],
                    ins=[b1t[:].opt()], outs=[ag[:].opt()])

            # =================== dense phase 2 ===================
            for t in range(dt_ if PH >= 4 else 0):
                if t % SG == 0:
                    ad_group_start(t // SG)
                m = min(128, n - t * 128)
                xt2 = dp.tile([128, 2, 128], f32, tag="xt2")
                pieces = []
                n0 = t * 128
                while n0 < t * 128 + m:
                    cb = n0 // nloc
                    ln = min(t * 128 + m - n0, (cb + 1) * nloc - n0)
                    pieces.append((n0, cb, n0 - cb * nloc, ln))
                    n0 += ln
                for h in range(2):
                    for (gn, cb, col, ln) in pieces:
                        off = gn - t * 128
                        nc.sync.dma_start(
                            xt2[:, h, off:off + ln],
                            ag[:][cb * 256 + h * 128:cb * 256 + h * 128 + 128,
                                  col:col + ln])
                ps = psd.tile([128, D1 + 8], f32, tag="psd")
                for h in range(2):
                    nc.tensor.matmul(ps[0:m, 0:C2 + 2], xt2[:, h, 0:m],
                                     w2sb[:, h, :], start=(h == 0), stop=(h == 1))
                st = sp.tile([128, T2W], f32, tag="st2")
                nc.gpsimd.memset(st[0:m, C2 + 2:T2W], 0.0)
                nc.scalar.activation(st[0:m, 0:C2 + 2], ps[0:m, 0:C2 + 2], AF.Copy)
                nc.vector.tensor_copy(adst_cur[0][0:m, t % SG, 0:1],
                                      ps[0:m, C2 + 1:C2 + 2])
                nc.sync.dma_start(t2[:][t * 128:t * 128 + m, :], st[0:m, :])
                if t % SG == SG - 1 or t == dt_ - 1:
                    ad_group_end(t // SG, ad2)

            # =================== layer-2 gather/aggregate ===================
            if PH >= 5:
                _gather_layer(nc, meta, mp, gp, np_, psa, pst, layer=2, tab=t2,
                              adt=ad2, idx_in=idx1_in, adidx_in=adidx_in,
                              dstl_in=dstl_in, iota=iota, ident=ident,
                              bias=b2sb, bounce=None, out_loc=out_loc)
            else:
                for t in range(-(-nloc // 128)):
                    m = min(128, nloc - t * 128)
                    nc.sync.dma_start(out_loc[:][t * 128:t * 128 + m, :],
                                      zeros[0:m, 0:C2])

    nc.compile()
    return nc


def _gather_layer(nc, meta, mp, gp, np_, psa, pst, *, layer, tab, adt, idx_in,
                  adidx_in, dstl_in, iota, ident, bias, bounce, out_loc):
    import os
    SUB = int(os.environ.get("GAT_L1SUB", "4"))
    n, nloc, split = meta["n"], meta["nloc"], meta["split"]
    nb = meta["nb"]
    nb = min(nb, int(os.environ.get("GAT_NB", str(nb))))
    W = T1W if layer == 1 else T2W
    D = D1 if layer == 1 else C2
    H = H1 if layer == 1 else 1
    CH = D // H
    psum_tiles = {}
    first_last = {t: (s[0], s[-1]) for t, s in enumerate(meta["tile_slots"]) if s}

    for bi in range(nb):
        kb = meta["k_b"][bi]
        blk = meta["batches"][bi]
        msg = mp.tile([128, B, W], f32, tag="msg")
        S = gp.tile([128, B, 128], f32, tag="S")
        adb = gp.tile([128, B, ADW], f32, tag="adb")
        u = gp.tile([128, B, H1], f32, tag="u")
        ut = gp.tile([128, B, H1], f32, tag="ut")
        idxs = gp.tile([128, B * 8], i16, tag="idxs")
        adix = gp.tile([128, B * 8], i16, tag="adix")
        dstl = gp.tile([128, B], f32, tag="dstl")
        nc.sync.dma_start(idxs[:], idx_in[:][bi])
        nc.sync.dma_start(adix[:], adidx_in[:][bi])
        nc.sync.dma_start(dstl[:], dstl_in[:][bi])

        G = os.environ.get("GAT_G", "all")
        if G in ("all", "fe"):
            if kb > 0:
                nc.gpsimd.dma_gather(
                    out_ap=msg[:, 0:kb, :], in_ap=tab[:][0:split, :],
                    idxs_ap=idxs[:, 0:kb * 8], num_idxs=kb * 128,
                    num_idxs_reg=kb * 128, elem_size=W, single_packet=False)
            if kb < B:
                nc.gpsimd.dma_gather(
                    out_ap=msg[:, kb:B, :], in_ap=tab[:][split:n, :],
                    idxs_ap=idxs[:, kb * 8:B * 8], num_idxs=(B - kb) * 128,
                    num_idxs_reg=(B - kb) * 128, elem_size=W,
                    single_packet=False)
        else:
            nc.vector.memset(msg[:], 0.0)
        if G in ("all", "ad"):
            nc.gpsimd.dma_gather(
                out_ap=adb[:], in_ap=adt[:], idxs_ap=adix[:],
                num_idxs=B * 128, num_idxs_reg=B * 128, elem_size=ADW,
                single_packet=False)
        else:
            nc.vector.memset(adb[:], 0.0)

        if SUB < 2:
            continue
        pm = msg[:].ap[0][0]
        pu = u[:].ap[0][0]
        pa = adb[:].ap[0][0]
        pd = dstl[:].ap[0][0]
        uH = u[:, :, 0:H]
        # u = exp(leakyrelu(a_s + a_d))
        nc.vector.tensor_add(
            uH, _ap(msg[:], [[pm, 128], [W, B], [1, H]], off=D),
            _ap(adb[:], [[pa, 128], [ADW, B], [1, H]]))
        nc.vector.tensor_scalar_mul(ut[:, :, 0:H], uH, NEG_SLOPE)
        nc.vector.tensor_max(uH, uH, ut[:, :, 0:H])
        nc.scalar.activation(uH, uH, AF.Exp)
        # scale features by u (broadcast over channels), in place
        nc.vector.tensor_tensor(
            _ap(msg[:], [[pm, 128], [W, B], [CH, H], [1, CH]]),
            _ap(msg[:], [[pm, 128], [W, B], [CH, H], [1, CH]]),
            _ap(u[:], [[pu, 128], [H1, B], [1, H], [0, CH]]), ALU.mult)
        # u into msg cols D:D+H (denominator rhs columns)
        nc.vector.tensor_copy(_ap(msg[:], [[pm, 128], [W, B], [1, H]], off=D), uH)
        # one-hot S
        nc.vector.tensor_tensor(
            S[:], _ap(dstl[:], [[pd, 128], [1, B], [0, 128]]),
            _ap(iota[:], [[iota[:].ap[0][0], 128], [0, B], [1, 128]]),
            ALU.is_equal)

        for s, (t, _hi) in enumerate(blk):
            if t < 0 or SUB < 3:
                continue
            fl = first_last[t]
            if fl[0] == (bi, s):
                psum_tiles[t] = psa.tile([128, D1 + H1], f32, tag="acc",
                                         name=f"acc_t{t}_l{layer}")
            acc = psum_tiles[t]
            nc.tensor.matmul(acc[:, 0:D + H], S[:, s, :], msg[:, s, 0:D + H],
                             start=(fl[0] == (bi, s)), stop=(fl[1] == (bi, s)))
            if fl[1] != (bi, s):
                continue
            if SUB < 4:
                del psum_tiles[t]
                continue
            # -------- finalize tile t --------
            m = min(128, nloc - t * 128)
            r = np_.tile([128, H1], f32, tag="recip")
            o = np_.tile([128, D1], f32, tag="o")
            oD = o[:, 0:D]
            nc.vector.reciprocal(r[:, 0:H], acc[:, D:D + H])
            po = o[:].ap[0][0]
            pr = r[:].ap[0][0]
            pacc = acc[:].ap[0][0]
            nc.vector.tensor_tensor(
                _ap(o[:], [[po, 128], [CH, H], [1, CH]]),
                _ap(acc[:], [[pacc, 128], [CH, H], [1, CH]]),
                _ap(r[:], [[pr, 128], [1, H], [0, CH]]), ALU.mult)
            nc.vector.tensor_add(oD, oD, bias[:, 0:D])
            if layer == 1:
                neg = np_.tile([128, D1], f32, tag="neg")
                nc.vector.tensor_scalar_min(neg[:], o[:], 0.0)
                nc.scalar.activation(neg[:], neg[:], AF.Exp)
                nc.vector.tensor_scalar_max(o[:], o[:], 0.0)
                nc.vector.tensor_add(o[:], o[:], neg[:])
                nc.vector.tensor_scalar_add(o[:], o[:], -1.0)
                for h in range(2):
                    pt = pst.tile([128, 128], f32, tag="tr")
                    nc.tensor.transpose(pt[:], o[:, h * 128:(h + 1) * 128],
                                        ident[:])
                    tr = np_.tile([128, 128], f32, tag="trsb")
                    nc.scalar.activation(tr[:], pt[:], AF.Copy)
                    nc.sync.dma_start(
                        bounce[:][h * 128:(h + 1) * 128, t * 128:t * 128 + m],
                        tr[:, 0:m])
            else:
                nc.sync.dma_start(out_loc[:][t * 128:t * 128 + m, :], oD[0:m, :])
            del psum_tiles[t]


# ============================ host-side driver ============================

_CACHE = {}


def _host_inputs(inputs, data, core):
    x = np.asarray(inputs["x"], np.float32)
    W1 = np.asarray(inputs["W1"], np.float32)
    W2 = np.asarray(inputs["W2"], np.float32)
    as1 = np.asarray(inputs["att_src1"], np.float32)
    ad1 = np.asarray(inputs["att_dst1"], np.float32)
    as2 = np.asarray(inputs["att_src2"], np.float32)
    ad2 = np.asarray(inputs["att_dst2"], np.float32)
    b1 = np.asarray(inputs["b1"], np.float32)
    b2 = np.asarray(inputs["b2"], np.float32)

    Was1 = np.einsum("fhc,hc->fh", W1.reshape(F_IN, H1, C1), as1)
    Wad1 = np.einsum("fhc,hc->fh", W1.reshape(F_IN, H1, C1), ad1)
    w1c = np.concatenate([W1, Was1, Wad1], axis=1).astype(np.float32)
    Was2 = (W2 @ as2[0][:, None]).astype(np.float32)
    Wad2 = (W2 @ ad2[0][:, None]).astype(np.float32)
    w2cat = np.concatenate([W2, Was2, Wad2], axis=1)          # [256, 66]
    w2c = np.ascontiguousarray(
        w2cat.reshape(2, 128, C2 + 2).transpose(1, 0, 2)).astype(np.float32)

    return {
        "xT": np.ascontiguousarray(x.T),
        "w1c": w1c,
        "w2c": w2c,
        "b1r": np.tile(b1[None, :], (128, 1)).astype(np.float32),
        "b2r": np.tile(b2[None, :], (128, 1)).astype(np.float32),
        "iota_in": np.tile(np.arange(128, dtype=np.float32), (128, 1)),
        "ident_in": np.eye(128, dtype=np.float32),
        "idx1": data["idx1"][core],
        "adidx": data["adidx"][core],
        "dstl": data["dstloc"][core],
        "adsc": data["adsc"][core],
    }


def _get_program(edge_index):
    key = hash(np.asarray(edge_index).tobytes())
    if key not in _CACHE:
        meta, data = _preprocess(edge_index)
        nc = _build(meta)
        _CACHE[key] = (meta, data, nc)
    return _CACHE[key]


def kernel(**inputs):
    meta, data, nc = _get_program(inputs["edge_index"])
    ncores = meta["ncores"]
    in_maps = [_host_inputs(inputs, data, c) for c in range(ncores)]
    res = bass_utils.run_bass_kernel_spmd(nc, in_maps, core_ids=list(range(ncores)))
    out = np.concatenate([res.results[c]["out_local"] for c in range(ncores)],
                         axis=0)
    return np.asarray(out, np.float32)


if __name__ == "__main__":
    rng = np.random.default_rng(0)
    ei = rng.integers(0, N, (2, 800000))
    meta, data = _preprocess(ei)
    nch = sum(len(s) for s in meta["tile_slots"])
    print("nb:", meta["nb"], "ntiles:", meta["ntiles"], "chunks:", nch,
          "pad_frac:", nch * 128 / (850000 / 8) - 1)


# revision 20
# speedup vs baseline: 1.1712x; 1.1712x over previous
"""2-layer GAT on 8 Trainium2 NeuronCores (Bass/Tile).

Sharding: dst nodes partitioned into 8 contiguous ranges (6250 each); each core
owns its range's incoming edges (host-sorted by dst). Dense projections are
computed redundantly per core into a per-core DRAM feature table; per-edge
source rows are fetched with dma_gather (int16 indices -> table split in two
halves). Segment softmax skips the max-subtraction (exactly cancels; the 1e-16
eps is negligible since every node has a self-loop). Aggregation: per 128-edge
chunk a one-hot S[p,m]=(dstloc[p]==m) is built on the vector engine and a PE
matmul S.T @ [u*h | u] accumulates numerator and denominator in PSUM. Layer-1
output (post bias+ELU) is transposed on-chip and AllGathered across the 8
cores; layer 2 repeats the same pipeline with one head.

kernel(**inputs) takes the full unsharded inputs and returns [50000, 64] f32.
"""
import numpy as np

import concourse.bass as bass
import concourse.bacc as bacc
import concourse.mybir as mybir
import concourse.tile as tile
from concourse import bass_utils

f32 = mybir.dt.float32
i16 = mybir.dt.int16
i32 = mybir.dt.int32
AF = mybir.ActivationFunctionType
ALU = mybir.AluOpType

NEG_SLOPE = 0.2

# problem geometry (graded problem)
N = 50000
F_IN = 128
H1, C1 = 4, 64
D1 = H1 * C1            # 256
C2 = 64
NCORES = 8
T1W = 320               # t1 row: h1[0:256], a_s1[256:260], a_d1[260:264], pad
T2W = 128               # t2 row: h2[0:64], a_s2[64], a_d2[65], pad
ADW = 64                # ad-table row width (256B = min gather granularity)
B = 32                  # chunks per batch (128 edges per chunk)
BIGIDX = 10**6


# ============================ host preprocessing ============================

def _preprocess(edge_index, n=N, ncores=NCORES):
    nloc = n // ncores
    split = n // 2
    src = np.asarray(edge_index[0], np.int64)
    dst = np.asarray(edge_index[1], np.int64)
    loop = np.arange(n, dtype=np.int64)
    src = np.concatenate([src, loop])
    dst = np.concatenate([dst, loop])
    order = np.argsort(dst, kind="stable")
    src, dst = src[order], dst[order]
    cuts = np.searchsorted(dst, np.arange(ncores + 1) * nloc)

    ntiles = -(-nloc // 128)
    per = [[None] * ntiles for _ in range(ncores)]
    for c in range(ncores):
        s, e = cuts[c], cuts[c + 1]
        cs, cd = src[s:e], dst[s:e] - c * nloc
        tcuts = np.searchsorted(cd, np.arange(ntiles + 1) * 128)
        for t in range(ntiles):
            a, b = tcuts[t], tcuts[t + 1]
            ts, td = cs[a:b], cd[a:b] - t * 128
            m = ts < split
            per[c][t] = (ts[m], td[m], ts[~m] - split, td[~m])

    # uniform per-(tile, side) chunk counts = max over cores
    nlo = [max(-(-len(per[c][t][0]) // 128) for c in range(ncores))
           for t in range(ntiles)]
    nhi = [max(-(-len(per[c][t][2]) // 128) for c in range(ncores))
           for t in range(ntiles)]

    seq = []
    for t in range(ntiles):
        seq += [(t, 0)] * nlo[t] + [(t, 1)] * nhi[t]
    nch = len(seq)
    nb = -(-nch // B)
    seq += [(-1, 0)] * (nb * B - nch)

    batches = []
    for bi in range(nb):
        blk = seq[bi * B:(bi + 1) * B]
        batches.append([x for x in blk if x[1] == 0] + [x for x in blk if x[1] == 1])
    k_b = [sum(1 for x in blk if x[1] == 0) for blk in batches]

    tile_slots = [[] for _ in range(ntiles)]
    for bi, blk in enumerate(batches):
        for s, (t, _) in enumerate(blk):
            if t >= 0:
                tile_slots[t].append((bi, s))

    def wrap(a):        # [k*128] -> [16, k*8] -> replicate to [128, k*8]
        return np.tile(a.reshape(-1, 16).T, (8, 1))

    # sub-gather splits: <=GSUB chunks (<=1024 idxs) per dma_gather call with
    # single_packet=True. Per batch: lo sub-calls then hi sub-calls.
    GSUB = 8
    subcalls = []   # per batch: list of (start_chunk, n_chunks, is_hi)
    for bi in range(nb):
        kb = k_b[bi]
        subs = []
        c0 = 0
        while c0 < kb:
            nsub = min(GSUB, kb - c0)
            subs.append((c0, nsub, 0))
            c0 += nsub
        while c0 < B:
            nsub = min(GSUB, B - c0)
            subs.append((c0, nsub, 1))
            c0 += nsub
        subcalls.append(subs)

    idx1 = np.zeros((ncores, nb, 128, B * 8), np.int16)
    dstloc = np.full((ncores, nb, 128, B), -1.0, np.float32)

    for c in range(ncores):
        cur = {}
        for bi, blk in enumerate(batches):
            gall = np.zeros((B, 128), np.int16)
            for s, (t, hi) in enumerate(blk):
                if t < 0:
                    continue
                j = cur.get((t, hi), 0)
                cur[(t, hi)] = j + 1
                lo_s, lo_d, hi_s, hi_d = per[c][t]
                es, ed = (hi_s, hi_d) if hi else (lo_s, lo_d)
                a, b2 = j * 128, min((j + 1) * 128, len(es))
                ln = max(0, b2 - a)
                if ln > 0:
                    gall[s, :ln] = es[a:b2]
                    dstloc[c, bi, :ln, s] = ed[a:b2]
            blocks = [wrap(gall[c0:c0 + ns].reshape(-1))
                      for (c0, ns, _) in subcalls[bi]]
            idx1[c, bi] = np.concatenate(blocks, axis=1)
    # partition-broadcast source: dstloc transposed [nb, B, 128]
    dstlT = np.ascontiguousarray(dstloc.transpose(0, 1, 3, 2))

    # scatter-add indices for the local ad tables: per group of SG dense
    # tiles, 128*SG wrapped int16 indices; non-local nodes go to trash row
    # `nloc`. idx position i <-> (partition i%128, slot i//128).
    SG = 8
    dt_ = -(-n // 128)
    nsg = -(-dt_ // SG)
    adsc = np.full((ncores, nsg, 128, SG * 8), 0, np.int16)
    for c in range(ncores):
        for g in range(nsg):
            vals = np.full(SG * 128, nloc, np.int64)
            for sl in range(SG):
                t = g * SG + sl
                if t >= dt_:
                    continue
                nodes = np.arange(t * 128, t * 128 + 128)
                loc = nodes - c * nloc
                ok = (loc >= 0) & (loc < nloc) & (nodes < n)
                i0 = sl * 128
                vals[i0:i0 + 128][ok] = loc[ok]
            adsc[c, g] = np.tile(vals.reshape(-1, 16).T, (8, 1))

    meta = dict(nb=nb, k_b=k_b, batches=batches, tile_slots=tile_slots,
                ntiles=ntiles, nloc=nloc, n=n, split=split, dt=dt_,
                ncores=ncores, sg=SG, nsg=nsg, subcalls=subcalls, gsub=GSUB)
    data = dict(idx1=idx1, dstloc=dstloc, dstlT=dstlT, adsc=adsc)
    return meta, data


# ============================ device program ============================

def _install_swdge_lane_patch():
    import concourse.tile_sem_assignment as tsa
    if getattr(tsa, "_gat_lane_patch", False):
        return
    orig = tsa.TileClockTick._assign_tick

    def _assign_tick(self, inst):
        qn = getattr(inst, "queue_num", None)
        if (qn is not None and inst.engine == mybir.EngineType.Pool
                and isinstance(inst, tsa.DMAInst)):
            tog = getattr(self, "_gat_qtoggle", None)
            if tog is None:
                tog = self._gat_qtoggle = {}
            t = tog.get(qn, 0)
            tog[qn] = t ^ 1
            lane = (qn * 2 + t) % self.swdge_sem_count
            self.next_sw_dma_idx = lane
        return orig(self, inst)

    tsa.TileClockTick._assign_tick = _assign_tick
    tsa._gat_lane_patch = True



def _ap(t_ap, dims, off=0):
    return bass.AP(t_ap.tensor, t_ap.offset + off, dims)


def _build(meta):
    import os
    PH = int(os.environ.get("GAT_PHASES", "5"))
    n, nloc, split, ncores = meta["n"], meta["nloc"], meta["split"], meta["ncores"]
    nb, dt_ = meta["nb"], meta["dt"]

    _install_swdge_lane_patch()
    nc = bacc.Bacc("TRN2", target_bir_lowering=False, debug=False,
                   num_devices=ncores, num_swdge_queues=4)

    xT = nc.dram_tensor("xT", [F_IN, n], f32, kind="ExternalInput")
    w1c = nc.dram_tensor("w1c", [F_IN, D1 + 8], f32, kind="ExternalInput")
    w2c = nc.dram_tensor("w2c", [128, 2, C2 + 2], f32, kind="ExternalInput")
    b1r = nc.dram_tensor("b1r", [128, D1], f32, kind="ExternalInput")
    b2r = nc.dram_tensor("b2r", [128, C2], f32, kind="ExternalInput")
    iota_in = nc.dram_tensor("iota_in", [128, 128], f32, kind="ExternalInput")
    iotac_in = nc.dram_tensor("iotac_in", [128, 1], f32, kind="ExternalInput")
    ident_in = nc.dram_tensor("ident_in", [128, 128], f32, kind="ExternalInput")
    idx1_in = nc.dram_tensor("idx1", [nb, 128, B * 8], i16, kind="ExternalInput")
    dstl_in = nc.dram_tensor("dstl", [nb, 128, B], f32, kind="ExternalInput")
    dstlT_in = nc.dram_tensor("dstlT", [nb, B, 128], f32, kind="ExternalInput")
    adsc_in = nc.dram_tensor("adsc", [meta["nsg"], 128, meta["sg"] * 8], i16,
                             kind="ExternalInput")
    out_loc = nc.dram_tensor("out_local", [nloc, C2], f32, kind="ExternalOutput")

    t1 = nc.dram_tensor("t1", [n, T1W], f32, kind="Internal")
    t2 = nc.dram_tensor("t2", [n, T2W], f32, kind="Internal")
    ad1 = nc.dram_tensor("ad1", [nloc + 1, ADW], f32, kind="Internal")
    ad2 = nc.dram_tensor("ad2", [nloc + 1, ADW], f32, kind="Internal")
    from concourse.replica_groups import maybe_share_collective_output_space
    ag_space = maybe_share_collective_output_space(
        "AllGather", [list(range(ncores))])
    b1t = nc.dram_tensor("b1t", [2 * 128, nloc], f32, kind="Internal")
    ag = nc.dram_tensor("ag", [ncores * 2 * 128, nloc], f32, kind="Internal",
                        addr_space=ag_space)

    with tile.TileContext(nc) as tc:
        with (
            tc.tile_pool(name="const", bufs=1) as cp,
            tc.tile_pool(name="dense", bufs=2) as dp,
            tc.tile_pool(name="stag", bufs=3) as sp,
            tc.tile_pool(name="adstag", bufs=1) as ap_,
            tc.tile_pool(name="msg", bufs=2) as mp,
            tc.tile_pool(name="gath", bufs=2) as gp,
            tc.tile_pool(name="norm", bufs=2) as np_,
            tc.tile_pool(name="psd", bufs=2, space="PSUM") as psd,
            tc.tile_pool(name="psa", bufs=4, space="PSUM") as psa,
            tc.tile_pool(name="pst", bufs=2, space="PSUM") as pst,
        ):
            zeros = cp.tile([128, 64], f32, tag="zeros")
            nc.vector.memset(zeros[:], 0.0)
            # zero-fill ad tables (only cols 0:4 are written by the scatter;
            # the gather reads whole 64-col rows)
            for adt_ in (ad1, ad2):
                nfull = ((nloc + 1) // 128) * 128
                if nfull > 0:
                    nc.sync.dma_start(
                        adt_[:][0:nfull, :],
                        _ap(zeros[:], [[zeros[:].ap[0][0], 128],
                                       [0, nfull // 128], [1, ADW]]))
                if nloc + 1 > nfull:
                    nc.sync.dma_start(adt_[:][nfull:nloc + 1, :],
                                      zeros[0:nloc + 1 - nfull, 0:ADW])
            w1sb = cp.tile([F_IN, D1 + 8], f32, tag="w1")
            w2sb = cp.tile([128, 2, C2 + 2], f32, tag="w2")
            b1sb = cp.tile([128, D1], f32, tag="b1")
            b2sb = cp.tile([128, C2], f32, tag="b2")
            iota = cp.tile([128, 128], f32, tag="iota")
            iotac = cp.tile([128, 1], f32, tag="iotac")
            ident = cp.tile([128, 128], f32, tag="ident")
            nc.sync.dma_start(w1sb[:], w1c[:])
            nc.sync.dma_start(w2sb[:], w2c[:])
            nc.sync.dma_start(b1sb[:], b1r[:])
            nc.sync.dma_start(b2sb[:], b2r[:])
            nc.sync.dma_start(iota[:], iota_in[:])
            nc.sync.dma_start(iotac[:], iotac_in[:])
            nc.sync.dma_start(ident[:], ident_in[:])

            # =================== dense phase 1 ===================
            SG = meta["sg"]
            nsg = meta["nsg"]
            adst_cur = [None]

            def ad_group_start(g):
                adst = ap_.tile([128, SG, ADW], f32, tag="adst",
                                name=f"adst_{g}")
                nc.vector.memset(adst[:], 0.0)
                adst_cur[0] = adst

            def ad_group_end(g, adt_):
                six = gp.tile([128, SG * 8], i16, tag="scix")
                nc.sync.dma_start(six[:], adsc_in[:][g])
                nc.gpsimd.dma_scatter_add(
                    out_ap=adt_[:], in_ap=adst_cur[0][:], idxs_ap=six[:],
                    num_idxs=SG * 128, num_idxs_reg=SG * 128, elem_size=ADW,
                    queue_num=g % 4)

            GD = 8
            for g in range(-(-dt_ // GD)):
                t0 = g * GD
                tn = min(GD, dt_ - t0)
                cols = min(tn * 128, n - t0 * 128)
                xt = dp.tile([F_IN, GD * 128], f32, tag="xt")
                nc.sync.dma_start(xt[:, 0:cols], xT[:, t0 * 128:t0 * 128 + cols])
                for t in range(t0, t0 + tn):
                    if t % SG == 0:
                        ad_group_start(t // SG)
                    m = min(128, n - t * 128)
                    ps = psd.tile([128, D1 + 8], f32, tag="psd")
                    nc.tensor.matmul(
                        ps[0:m, :], xt[:, (t - t0) * 128:(t - t0) * 128 + m],
                        w1sb[:], start=True, stop=True)
                    st = sp.tile([128, T1W], f32, tag="st1")
                    nc.gpsimd.memset(st[0:m, D1 + 8:T1W], 0.0)
                    nc.scalar.activation(st[0:m, 0:D1 + 8], ps[0:m, :], AF.Copy)
                    nc.vector.tensor_copy(adst_cur[0][0:m, t % SG, 0:4],
                                          ps[0:m, D1 + 4:D1 + 8])
                    nc.sync.dma_start(t1[:][t * 128:t * 128 + m, :], st[0:m, :])
                    if t % SG == SG - 1 or t == dt_ - 1:
                        ad_group_end(t // SG, ad1)

            # =================== layer-1 gather/aggregate ===================
            if PH >= 2:
                _gather_layer(nc, meta, mp, gp, np_, psa, psd, pst, layer=1,
                              tab=t1, adt=ad1, idx_in=idx1_in,
                              dstl_in=dstl_in, dstlT_in=dstlT_in, iota=iota,
                              iotac=iotac, ident=ident, bias=b1sb, bounce=b1t,
                              out_loc=None)

            # =================== AllGather ===================
            if PH >= 3:
                nc.gpsimd.collective_compute(
                    "AllGather", ALU.bypass,
                    replica_groups=[list(range(ncores))],
                    ins=[b1t[:].opt()], outs=[ag[:].opt()])

            # =================== dense phase 2 ===================
            for t in range(dt_ if PH >= 4 else 0):
                if t % SG == 0:
                    ad_group_start(t // SG)
                m = min(128, n - t * 128)
                xt2 = dp.tile([128, 2, 128], f32, tag="xt2")
                pieces = []
                n0 = t * 128
                while n0 < t * 128 + m:
                    cb = n0 // nloc
                    ln = min(t * 128 + m - n0, (cb + 1) * nloc - n0)
                    pieces.append((n0, cb, n0 - cb * nloc, ln))
                    n0 += ln
                for h in range(2):
                    for (gn, cb, col, ln) in pieces:
                        off = gn - t * 128
                        nc.sync.dma_start(
                            xt2[:, h, off:off + ln],
                            ag[:][cb * 256 + h * 128:cb * 256 + h * 128 + 128,
                                  col:col + ln])
                ps = psd.tile([128, D1 + 8], f32, tag="psd")
                for h in range(2):
                    nc.tensor.matmul(ps[0:m, 0:C2 + 2], xt2[:, h, 0:m],
                                     w2sb[:, h, :], start=(h == 0), stop=(h == 1))
                st = sp.tile([128, T2W], f32, tag="st2")
                nc.gpsimd.memset(st[0:m, C2 + 2:T2W], 0.0)
                nc.scalar.activation(st[0:m, 0:C2 + 2], ps[0:m, 0:C2 + 2], AF.Copy)
                nc.vector.tensor_copy(adst_cur[0][0:m, t % SG, 0:1],
                                      ps[0:m, C2 + 1:C2 + 2])
                nc.sync.dma_start(t2[:][t * 128:t * 128 + m, :], st[0:m, :])
                if t % SG == SG - 1 or t == dt_ - 1:
                    ad_group_end(t // SG, ad2)

            # =================== layer-2 gather/aggregate ===================
            if PH >= 5:
                _gather_layer(nc, meta, mp, gp, np_, psa, psd, pst, layer=2,
                              tab=t2, adt=ad2, idx_in=idx1_in,
                              dstl_in=dstl_in, dstlT_in=dstlT_in, iota=iota,
                              iotac=iotac, ident=ident, bias=b2sb, bounce=None,
                              out_loc=out_loc)
            else:
                for t in range(-(-nloc // 128)):
                    m = min(128, nloc - t * 128)
                    nc.sync.dma_start(out_loc[:][t * 128:t * 128 + m, :],
                                      zeros[0:m, 0:C2])

    nc.compile()
    return nc


def _gather_layer(nc, meta, mp, gp, np_, psa, psd, pst, *, layer, tab, adt,
                  idx_in, dstl_in, dstlT_in, iota, iotac, ident, bias, bounce,
                  out_loc):
    n, nloc, split = meta["n"], meta["nloc"], meta["split"]
    nb = meta["nb"]
    W = T1W if layer == 1 else T2W
    D = D1 if layer == 1 else C2
    H = H1 if layer == 1 else 1
    CH = D // H
    psum_tiles = {}
    ad_tiles = {}
    qrr = [0]

    def nextq():
        qrr[0] = (qrr[0] + 1) % 4
        return qrr[0]

    first_last = {t: (s[0], s[-1]) for t, s in enumerate(meta["tile_slots"]) if s}

    for bi in range(nb):
        kb = meta["k_b"][bi]
        blk = meta["batches"][bi]
        msg = mp.tile([128, B, W], f32, tag="msg")
        S = gp.tile([128, B, 128], f32, tag="S")
        ST = gp.tile([128, B, 128], f32, tag="ST")
        dstb = gp.tile([128, B * 128], f32, tag="dstb")
        u = gp.tile([128, B, H1], f32, tag="u")
        ut = gp.tile([128, B, H1], f32, tag="ut")
        idxs = gp.tile([128, B * 8], i16, tag="idxs")
        dstl = gp.tile([128, B], f32, tag="dstl")
        nc.sync.dma_start(idxs[:], idx_in[:][bi])
        nc.sync.dma_start(dstl[:], dstl_in[:][bi])
        # partition-replicated dstloc (chunk-major) for the S_T build
        nc.sync.dma_start(
            dstb[:], _ap(dstlT_in[:], [[0, 128], [1, B * 128]],
                         off=bi * B * 128))

        # feature gathers: <=1024-idx single-packet calls, round-robin queues
        col = 0
        for (c0, nch, hi) in meta["subcalls"][bi]:
            base = tab[:][split:n, :] if hi else tab[:][0:split, :]
            nc.gpsimd.dma_gather(
                out_ap=msg[:, c0:c0 + nch, :], in_ap=base,
                idxs_ap=idxs[:, col:col + nch * 8], num_idxs=nch * 128,
                num_idxs_reg=nch * 128, elem_size=W, queue_num=nextq())
            col += nch * 8

        pm = msg[:].ap[0][0]
        pu = u[:].ap[0][0]
        pd = dstl[:].ap[0][0]
        # one-hot S[p_edge, c, m_node] = (dstloc[p, c] == m)
        nc.vector.tensor_tensor(
            S[:], _ap(dstl[:], [[pd, 128], [1, B], [0, 128]]),
            _ap(iota[:], [[iota[:].ap[0][0], 128], [0, B], [1, 128]]),
            ALU.is_equal)
        # transposed one-hot S_T[m_node, c, p_edge] = (dstloc[p, c] == m)
        nc.vector.tensor_tensor(
            ST[:], _ap(dstb[:], [[dstb[:].ap[0][0], 128], [1, B * 128]]),
            _ap(iotac[:], [[iotac[:].ap[0][0], 128], [0, B * 128]]),
            ALU.is_equal)

        # per-chunk a_d broadcast: adp[:, c*4h] = S_T[:,c,:].T @ a_d_tile
        adp = psd.tile([128, B * H1], f32, tag="psd", name=f"adp_{layer}_{bi}")
        nc.vector.memset(adp, 0.0)
        for s, (t, _hi) in enumerate(blk):
            if t < 0:
                continue
            if t not in ad_tiles:
                adt_sb = gp.tile([128, ADW], f32, tag="adt", bufs=6,
                                 name=f"adt_{layer}_{t}")
                m = min(128, nloc - t * 128)
                if m < 128:
                    nc.vector.memset(adt_sb[:], 0.0)
                nc.sync.dma_start(adt_sb[0:m, :],
                                  adt[:][t * 128:t * 128 + m, :])
                ad_tiles[t] = adt_sb
            nc.tensor.matmul(adp[:, s * H1:s * H1 + H], ST[:, s, :],
                             ad_tiles[t][:, 0:H], start=True, stop=True)

        uH = u[:, :, 0:H]
        # u = exp(leakyrelu(a_s + a_d))
        nc.vector.tensor_add(
            uH, _ap(msg[:], [[pm, 128], [W, B], [1, H]], off=D),
            _ap(adp, [[adp.ap[0][0], 128], [H1, B], [1, H]]))
        nc.vector.tensor_scalar_mul(ut[:, :, 0:H], uH, NEG_SLOPE)
        nc.vector.tensor_max(uH, uH, ut[:, :, 0:H])
        nc.scalar.activation(uH, uH, AF.Exp)
        # scale features by u (broadcast over channels), in place
        nc.vector.tensor_tensor(
            _ap(msg[:], [[pm, 128], [W, B], [CH, H], [1, CH]]),
            _ap(msg[:], [[pm, 128], [W, B], [CH, H], [1, CH]]),
            _ap(u[:], [[pu, 128], [H1, B], [1, H], [0, CH]]), ALU.mult)
        # u into msg cols D:D+H (denominator rhs columns)
        nc.vector.tensor_copy(_ap(msg[:], [[pm, 128], [W, B], [1, H]], off=D), uH)

        for s, (t, _hi) in enumerate(blk):
            if t < 0:
                continue
            fl = first_last[t]
            if fl[0] == (bi, s):
                psum_tiles[t] = psa.tile([128, D1 + H1], f32, tag="acc",
                                         name=f"acc_t{t}_l{layer}")
            acc = psum_tiles[t]
            nc.tensor.matmul(acc[:, 0:D + H], S[:, s, :], msg[:, s, 0:D + H],
                             start=(fl[0] == (bi, s)), stop=(fl[1] == (bi, s)))
            if fl[1] != (bi, s):
                continue
            # -------- finalize tile t --------
            m = min(128, nloc - t * 128)
            r = np_.tile([128, H1], f32, tag="recip")
            o = np_.tile([128, D1], f32, tag="o")
            oD = o[:, 0:D]
            nc.vector.reciprocal(r[:, 0:H], acc[:, D:D + H])
            po = o[:].ap[0][0]
            pr = r[:].ap[0][0]
            pacc = acc[:].ap[0][0]
            nc.vector.tensor_tensor(
                _ap(o[:], [[po, 128], [CH, H], [1, CH]]),
                _ap(acc[:], [[pacc, 128], [CH, H], [1, CH]]),
                _ap(r[:], [[pr, 128], [1, H], [0, CH]]), ALU.mult)
            nc.vector.tensor_add(oD, oD, bias[:, 0:D])
            if layer == 1:
                neg = np_.tile([128, D1], f32, tag="neg")
                nc.vector.tensor_scalar_min(neg[:], o[:], 0.0)
                nc.scalar.activation(neg[:], neg[:], AF.Exp)
                nc.vector.tensor_scalar_max(o[:], o[:], 0.0)
                nc.vector.tensor_add(o[:], o[:], neg[:])
                nc.vector.tensor_scalar_add(o[:], o[:], -1.0)
                for h in range(2):
                    pt = pst.tile([128, 128], f32, tag="tr")
                    nc.tensor.transpose(pt[:], o[:, h * 128:(h + 1) * 128],
                                        ident[:])
                    tr = np_.tile([128, 128], f32, tag="trsb")
                    nc.scalar.activation(tr[:], pt[:], AF.Copy)
                    nc.sync.dma_start(
                        bounce[:][h * 128:(h + 1) * 128, t * 128:t * 128 + m],
                        tr[:, 0:m])
            else:
                nc.sync.dma_start(out_loc[:][t * 128:t * 128 + m, :], oD[0:m, :])
            del psum_tiles[t]
            del ad_tiles[t]


# ============================ host-side driver ============================

_CACHE = {}


def _host_inputs(inputs, data, core):
    x = np.asarray(inputs["x"], np.float32)
    W1 = np.asarray(inputs["W1"], np.float32)
    W2 = np.asarray(inputs["W2"], np.float32)
    as1 = np.asarray(inputs["att_src1"], np.float32)
    ad1 = np.asarray(inputs["att_dst1"], np.float32)
    as2 = np.asarray(inputs["att_src2"], np.float32)
    ad2 = np.asarray(inputs["att_dst2"], np.float32)
    b1 = np.asarray(inputs["b1"], np.float32)
    b2 = np.asarray(inputs["b2"], np.float32)

    Was1 = np.einsum("fhc,hc->fh", W1.reshape(F_IN, H1, C1), as1)
    Wad1 = np.einsum("fhc,hc->fh", W1.reshape(F_IN, H1, C1), ad1)
    w1c = np.concatenate([W1, Was1, Wad1], axis=1).astype(np.float32)
    Was2 = (W2 @ as2[0][:, None]).astype(np.float32)
    Wad2 = (W2 @ ad2[0][:, None]).astype(np.float32)
    w2cat = np.concatenate([W2, Was2, Wad2], axis=1)          # [256, 66]
    w2c = np.ascontiguousarray(
        w2cat.reshape(2, 128, C2 + 2).transpose(1, 0, 2)).astype(np.float32)

    return {
        "xT": np.ascontiguousarray(x.T),
        "w1c": w1c,
        "w2c": w2c,
        "b1r": np.tile(b1[None, :], (128, 1)).astype(np.float32),
        "b2r": np.tile(b2[None, :], (128, 1)).astype(np.float32),
        "iota_in": np.tile(np.arange(128, dtype=np.float32), (128, 1)),
        "iotac_in": np.arange(128, dtype=np.float32)[:, None].copy(),
        "ident_in": np.eye(128, dtype=np.float32),
        "idx1": data["idx1"][core],
        "dstl": data["dstloc"][core],
        "dstlT": data["dstlT"][core],
        "adsc": data["adsc"][core],
    }


def _get_program(edge_index):
    key = hash(np.asarray(edge_index).tobytes())
    if key not in _CACHE:
        meta, data = _preprocess(edge_index)
        nc = _build(meta)
        _CACHE[key] = (meta, data, nc)
    return _CACHE[key]


def kernel(**inputs):
    meta, data, nc = _get_program(inputs["edge_index"])
    ncores = meta["ncores"]
    in_maps = [_host_inputs(inputs, data, c) for c in range(ncores)]
    res = bass_utils.run_bass_kernel_spmd(nc, in_maps, core_ids=list(range(ncores)))
    out = np.concatenate([res.results[c]["out_local"] for c in range(ncores)],
                         axis=0)
    return np.asarray(out, np.float32)


if __name__ == "__main__":
    rng = np.random.default_rng(0)
    ei = rng.integers(0, N, (2, 800000))
    meta, data = _preprocess(ei)
    nch = sum(len(s) for s in meta["tile_slots"])
    print("nb:", meta["nb"], "ntiles:", meta["ntiles"], "chunks:", nch,
          "pad_frac:", nch * 128 / (850000 / 8) - 1)
